# revision 7
# baseline (speedup 1.0000x reference)
"""Fused transformer encoder layer (post-norm, 16 heads, d=1024, ff=4096)
for one full TRN2 chip (8 NeuronCores, SPMD, no collectives).

Sharding: core c handles batch b=c//2, query-half h=c%2 (1024 tokens).
Each core computes k/v for its whole batch sequence (2048 tokens, keys
reordered own-half-first -- softmax is permutation invariant over keys),
and q/attention/FFN/layernorms for its own 1024 tokens.

On-chip layout is feature-major (d on partitions, tokens on free dim).
Scores are computed transposed ([keys, queries]) so the exp output feeds
attn@V directly as the moving operand; softmax denominators come from a
ones-column appended to V (row 64 of the attn@V accumulation); the V
bias is folded into the output-projection bias host-side.

SBUF is tight, so one master pool time-multiplexes the big tensors via
explicit tags (slots rotate when the previous tenant's accessors finish):
  x1: xqbf -> attnT -> u1      x2: xrbf -> u2
  x3: wv   -> u3               x4: qT   -> u4
  kk: kT   -> LN scratch       vv: vext          xq: x own (f32, resident)
"""

import numpy as np
import ml_dtypes

import concourse.bass as bass
import concourse.mybir as mybir
import concourse.tile as tile
from concourse import bacc
from concourse import bass_utils

D = 1024       # d_model
H = 16         # heads
DH = 64        # head dim
FF = 4096      # d_ff
TQ = 1024      # query tokens per core
TK = 2048      # key tokens per core (full batch seq)
PD = 128       # partitions
NDT = D // PD  # 8 d-tiles
NKT = TK // PD # 16 key tiles
NFT = FF // PD # 32 ff tiles
TT = 512       # matmul moving free-dim tile
NQT = TQ // TT # 2 query tiles
EPS = 1e-5

F32 = mybir.dt.float32
F32R = mybir.dt.float32r
BF16 = mybir.dt.bfloat16
BF = ml_dtypes.bfloat16

AF = mybir.ActivationFunctionType
ALU = mybir.AluOpType

_CACHE = {}


def _build_nc():
    nc = bacc.Bacc("TRN2", target_bir_lowering=False)

    # ---- DRAM I/O ----
    d_xq32 = nc.dram_tensor("xq32t", [D, TQ], F32, kind="ExternalInput")
    d_xqbf = nc.dram_tensor("xqbft", [D, TQ], BF16, kind="ExternalInput")
    d_xrbf = nc.dram_tensor("xrbft", [D, TQ], BF16, kind="ExternalInput")
    d_wq = nc.dram_tensor("wqbf", [D, D], BF16, kind="ExternalInput")  # pre-scaled 1/8
    d_wk = nc.dram_tensor("wkbf", [D, D], BF16, kind="ExternalInput")
    d_wv = nc.dram_tensor("wvbf", [D, D], BF16, kind="ExternalInput")
    d_wo = nc.dram_tensor("wobf", [D, D], BF16, kind="ExternalInput")
    d_w1 = nc.dram_tensor("w1bf", [D, FF], BF16, kind="ExternalInput")
    d_w2 = nc.dram_tensor("w2bf", [FF, D], BF16, kind="ExternalInput")
    d_bq = nc.dram_tensor("bq8", [PD, NDT], F32, kind="ExternalInput")
    d_bk = nc.dram_tensor("bkp", [PD, NDT], F32, kind="ExternalInput")
    d_b1 = nc.dram_tensor("b1p", [PD, NFT], F32, kind="ExternalInput")
    d_bo2 = nc.dram_tensor("bo2row", [1, D], BF16, kind="ExternalInput")
    d_b2 = nc.dram_tensor("b2row", [1, D], BF16, kind="ExternalInput")
    d_g1 = nc.dram_tensor("g1p", [PD, NDT], F32, kind="ExternalInput")
    d_be1 = nc.dram_tensor("be1p", [PD, NDT], F32, kind="ExternalInput")
    d_g2 = nc.dram_tensor("g2p", [PD, NDT], F32, kind="ExternalInput")
    d_be2 = nc.dram_tensor("be2p", [PD, NDT], F32, kind="ExternalInput")
    d_yt = nc.dram_tensor("yt", [D, TQ], F32, kind="ExternalOutput")

    r_xq32 = d_xq32.rearrange("(dt p) t -> p dt t", p=PD)
    r_xqbf = d_xqbf.rearrange("(dt p) t -> p dt t", p=PD)
    r_xrbf = d_xrbf.rearrange("(dt p) t -> p dt t", p=PD)
    r_wq = d_wq.rearrange("(kt p) o -> p kt o", p=PD)
    r_wk = d_wk.rearrange("(kt p) o -> p kt o", p=PD)
    r_wv = d_wv.rearrange("(kt p) o -> p kt o", p=PD)
    r_wo = d_wo.rearrange("(kt p) o -> p kt o", p=PD)
    r_w1 = d_w1.rearrange("(kt p) f -> p kt f", p=PD)
    r_w2 = d_w2.rearrange("(ft p) o -> p ft o", p=PD)
    r_yt = d_yt.rearrange("(dt p) t -> p dt t", p=PD)

    with tile.TileContext(nc) as tc:
        with (
            tc.tile_pool(name="persist", bufs=1) as persist,
            tc.tile_pool(name="mp", bufs=1) as mp,
            tc.tile_pool(name="wpool", bufs=3) as wpool,
            tc.tile_pool(name="ppool", bufs=3) as ppool,
            tc.tile_pool(name="spool", bufs=2) as spool,
            tc.tile_pool(name="psA", bufs=2, space="PSUM") as psA,
            tc.tile_pool(name="psB", bufs=2, space="PSUM") as psB,
        ):
            # ---- constants / biases (persist) ----
            ones128 = persist.tile([PD, 1], BF16)
            onesrow = persist.tile([1, TT], BF16)
            bq_sb = persist.tile([PD, NDT], F32)
            bk_sb = persist.tile([PD, NDT], F32)
            b1_sb = persist.tile([PD, NFT], F32)
            bo2_sb = persist.tile([1, D], BF16)
            b2_sb = persist.tile([1, D], BF16)
            g1_sb = persist.tile([PD, NDT], F32)
            be1_sb = persist.tile([PD, NDT], F32)
            g2_sb = persist.tile([PD, NDT], F32)
            be2_sb = persist.tile([PD, NDT], F32)
            eps_sb = persist.tile([1, 1], F32)

            nc.vector.memset(ones128, 1.0)
            nc.vector.memset(onesrow, 1.0)
            nc.vector.memset(eps_sb, EPS)
            nc.sync.dma_start(out=bq_sb, in_=d_bq[:, :])
            nc.sync.dma_start(out=bk_sb, in_=d_bk[:, :])
            nc.sync.dma_start(out=b1_sb, in_=d_b1[:, :])
            nc.sync.dma_start(out=bo2_sb, in_=d_bo2[:, :])
            nc.sync.dma_start(out=b2_sb, in_=d_b2[:, :])
            nc.sync.dma_start(out=g1_sb, in_=d_g1[:, :])
            nc.sync.dma_start(out=be1_sb, in_=d_be1[:, :])
            nc.sync.dma_start(out=g2_sb, in_=d_g2[:, :])
            nc.sync.dma_start(out=be2_sb, in_=d_be2[:, :])

            # ---- big tensors, phase 1 tenants ----
            xq32 = mp.tile([PD, NDT, TQ], F32, tag="xq")    # x own -> h -> h1 -> y
            xqbf = mp.tile([PD, NDT, TQ], BF16, tag="x1")
            xrbf = mp.tile([PD, NDT, TQ], BF16, tag="x2")
            wv_sb = mp.tile([PD, NDT, D], BF16, tag="x3")
            qT = mp.tile([PD, NDT, TQ], BF16, tag="x4")
            kT = mp.tile([PD, NDT, TK], BF16, tag="kk")
            vext = mp.tile([PD, NKT, H * 65], BF16, tag="vv")  # [V_h | ones] per head

            for dt in range(NDT):
                nc.sync.dma_start(out=xq32[:, dt, :], in_=r_xq32[:, dt, :])
                nc.sync.dma_start(out=xqbf[:, dt, :], in_=r_xqbf[:, dt, :])
                nc.sync.dma_start(out=xrbf[:, dt, :], in_=r_xrbf[:, dt, :])
                nc.sync.dma_start(out=wv_sb[:, dt, :], in_=r_wv[:, dt, :])

            # ones columns of vext
            for h in range(H):
                nc.vector.memset(vext[:, :, h * 65 + 64 : h * 65 + 65], 1.0)

            # ================= projections =================
            # q^T (weight-stationary): q = (x@Wq + bq)/8, 1/8 folded into Wq/bq
            for o in range(NDT):
                wq_t = wpool.tile([PD, NDT, PD], BF16, tag="w")
                nc.sync.dma_start(out=wq_t, in_=r_wq[:, :, o * PD : (o + 1) * PD])
                ps = psA.tile([PD, TQ], F32, tag="a")
                for k in range(NDT):
                    for t in range(NQT):
                        nc.tensor.matmul(
                            ps[:, t * TT : (t + 1) * TT],
                            lhsT=wq_t[:, k, :],
                            rhs=xqbf[:, k, t * TT : (t + 1) * TT],
                            start=(k == 0),
                            stop=(k == NDT - 1),
                        )
                nc.vector.tensor_scalar_add(qT[:, o, :], ps, bq_sb[:, o : o + 1])

            # k^T for all 2048 keys (own tokens first, then rest)
            for o in range(NDT):
                wk_t = wpool.tile([PD, NDT, PD], BF16, tag="w")
                nc.sync.dma_start(out=wk_t, in_=r_wk[:, :, o * PD : (o + 1) * PD])
                for half, xsrc in ((0, xqbf), (1, xrbf)):
                    ps = psA.tile([PD, TQ], F32, tag="a")
                    for k in range(NDT):
                        for t in range(NQT):
                            nc.tensor.matmul(
                                ps[:, t * TT : (t + 1) * TT],
                                lhsT=wk_t[:, k, :],
                                rhs=xsrc[:, k, t * TT : (t + 1) * TT],
                                start=(k == 0),
                                stop=(k == NDT - 1),
                            )
                    nc.vector.tensor_scalar_add(
                        kT[:, o, half * TQ : (half + 1) * TQ],
                        ps,
                        bk_sb[:, o : o + 1],
                    )

            # v token-major (activation-stationary), no bias (folded into bo2)
            for tt in range(NKT):
                xsrc = xqbf if tt < NDT else xrbf
                ti = tt % NDT
                ps = psA.tile([PD, TQ], F32, tag="a")
                for k in range(NDT):
                    for half in range(2):
                        nc.tensor.matmul(
                            ps[:, half * TT : (half + 1) * TT],
                            lhsT=xsrc[:, k, ti * PD : (ti + 1) * PD],
                            rhs=wv_sb[:, k, half * TT : (half + 1) * TT],
                            start=(k == 0),
                            stop=(k == NDT - 1),
                        )
                # scatter heads into the 65-stride layout
                nc.vector.tensor_copy(
                    vext[:, tt, :].rearrange("p (h e) -> p h e", e=65)[:, :, 0:64],
                    ps.rearrange("p (h e) -> p h e", e=64),
                )

            # ================= attention =================
            # attnT: head pair (2i, 2i+1) at dt=i, partitions [0:64] / [64:128]
            attnT = mp.tile([PD, NDT, TQ], BF16, tag="x1")
            for h in range(H):
                hp = (h % 2) * 64
                hd = h // 2
                pso = psB.tile([65, TQ], F32, tag="b")
                for kt in range(NKT):
                    pss = psA.tile([PD, TQ], F32, tag="a")
                    for t in range(NQT):
                        nc.tensor.matmul(
                            pss[:, t * TT : (t + 1) * TT],
                            lhsT=kT[hp : hp + 64, hd, kt * PD : (kt + 1) * PD],
                            rhs=qT[hp : hp + 64, hd, t * TT : (t + 1) * TT],
                            start=True,
                            stop=True,
                        )
                    pt = ppool.tile([PD, TQ], BF16, tag="pt")
                    nc.scalar.activation(pt, pss, AF.Exp)
                    for t in range(NQT):
                        nc.tensor.matmul(
                            pso[:, t * TT : (t + 1) * TT],
                            lhsT=vext[:, kt, h * 65 : h * 65 + 65],
                            rhs=pt[:, t * TT : (t + 1) * TT],
                            start=(kt == 0),
                            stop=(kt == NKT - 1),
                        )
                recip = spool.tile([1, TQ], F32, tag="recip")
                nc.vector.reciprocal(recip, pso[64:65, :])
                bc = spool.tile([64, TQ], F32, tag="bc")
                nc.gpsimd.partition_broadcast(bc, recip)
                if h % 2 == 0:
                    nc.vector.tensor_mul(attnT[0:64, hd, :], pso[0:64, :], bc)
                else:
                    nrm = spool.tile([64, TQ], BF16, tag="nrm")
                    nc.vector.tensor_mul(nrm, pso[0:64, :], bc)
                    nc.sync.dma_start(out=attnT[64:128, hd, :], in_=nrm)

            # ================= out-projection + residual =================
            for o in range(NDT):
                wo_t = wpool.tile([PD, NDT, PD], BF16, tag="w")
                nc.sync.dma_start(out=wo_t, in_=r_wo[:, :, o * PD : (o + 1) * PD])
                ps = psA.tile([PD, TQ], F32, tag="a")
                for k in range(NDT):
                    for t in range(NQT):
                        nc.tensor.matmul(
                            ps[:, t * TT : (t + 1) * TT],
                            lhsT=wo_t[:, k, :],
                            rhs=attnT[:, k, t * TT : (t + 1) * TT],
                            start=(k == 0),
                            stop=False,
                        )
                for t in range(NQT):
                    nc.tensor.matmul(
                        ps[:, t * TT : (t + 1) * TT],
                        lhsT=bo2_sb[:, o * PD : (o + 1) * PD],
                        rhs=onesrow[:, 0:TT],
                        start=False,
                        stop=(t == NQT - 1),
                    )
                nc.vector.tensor_add(xq32[:, o, :], xq32[:, o, :], ps)

            # ================= layernorm (stats via ones-matmul) =================
            lnb = mp.tile([PD, 4, TQ], F32, tag="kk")  # mu_b, rstd_b, mu, tmp
            sbf = mp.tile([PD, NDT + 1, TQ], BF16, tag="vv")  # bf16 x-copy + sq

            def layernorm(g_sb, be_sb):
                pstat_h = psA.tile([1, TQ], F32, tag="a")
                pstat_h2 = psA.tile([1, TQ], F32, tag="a")
                for k in range(NDT):
                    nc.vector.tensor_copy(sbf[:, k, :], xq32[:, k, :])
                    for t in range(NQT):
                        nc.tensor.matmul(
                            pstat_h[:, t * TT : (t + 1) * TT],
                            lhsT=ones128,
                            rhs=sbf[:, k, t * TT : (t + 1) * TT],
                            start=(k == 0),
                            stop=(k == NDT - 1),
                        )
                for k in range(NDT):
                    sq = sbf[:, NDT, :]
                    nc.vector.tensor_mul(sq, sbf[:, k, :], sbf[:, k, :])
                    for t in range(NQT):
                        nc.tensor.matmul(
                            pstat_h2[:, t * TT : (t + 1) * TT],
                            lhsT=ones128,
                            rhs=sq[:, t * TT : (t + 1) * TT],
                            start=(k == 0),
                            stop=(k == NDT - 1),
                        )
                mu = lnb[0:1, 2, :]
                tmp = lnb[0:1, 3, :]
                nc.vector.tensor_scalar_mul(mu, pstat_h, 1.0 / D)
                nc.vector.tensor_mul(tmp, mu, mu)
                # tmp = E[x^2] - mu^2
                nc.vector.scalar_tensor_tensor(
                    out=tmp,
                    in0=pstat_h2,
                    scalar=1.0 / D,
                    in1=tmp,
                    op0=ALU.mult,
                    op1=ALU.subtract,
                )
                nc.scalar.activation(tmp, tmp, AF.Sqrt, bias=eps_sb[:, 0:1])
                nc.vector.reciprocal(tmp, tmp)
                mu_b = lnb[:, 0, :]
                rstd_b = lnb[:, 1, :]
                nc.gpsimd.partition_broadcast(mu_b, mu)
                nc.gpsimd.partition_broadcast(rstd_b, tmp)
                for k in range(NDT):
                    nc.vector.tensor_sub(xq32[:, k, :], xq32[:, k, :], mu_b)
                    nc.vector.scalar_tensor_tensor(
                        out=xq32[:, k, :],
                        in0=xq32[:, k, :],
                        scalar=g_sb[:, k : k + 1],
                        in1=rstd_b,
                        op0=ALU.mult,
                        op1=ALU.mult,
                    )
                    nc.vector.tensor_scalar_add(
                        xq32[:, k, :], xq32[:, k, :], be_sb[:, k : k + 1]
                    )

            layernorm(g1_sb, be1_sb)

            # ================= FFN =================
            u_parts = [
                mp.tile([PD, NFT // 4, TQ], BF16, tag=t4, name=f"u{i}")
                for i, t4 in enumerate(("x2", "x3", "x4", "x1"))
            ]

            def u_slice(ft, tsl):
                return u_parts[ft // (NFT // 4)][:, ft % (NFT // 4), tsl]

            for k in range(NDT):
                nc.vector.tensor_copy(sbf[:, k, :], xq32[:, k, :])
            for ft in range(NFT):
                w1_t = wpool.tile([PD, NDT, PD], BF16, tag="w")
                nc.sync.dma_start(out=w1_t, in_=r_w1[:, :, ft * PD : (ft + 1) * PD])
                ps = psA.tile([PD, TQ], F32, tag="a")
                for k in range(NDT):
                    for t in range(NQT):
                        nc.tensor.matmul(
                            ps[:, t * TT : (t + 1) * TT],
                            lhsT=w1_t[:, k, :],
                            rhs=sbf[:, k, t * TT : (t + 1) * TT],
                            start=(k == 0),
                            stop=(k == NDT - 1),
                        )
                # u = relu(ps + b1)
                nc.vector.tensor_scalar(
                    u_slice(ft, slice(None)),
                    ps,
                    b1_sb[:, ft : ft + 1],
                    0.0,
                    ALU.add,
                    ALU.max,
                )
            for o in range(NDT):
                w2a = wpool.tile([PD, NFT // 2, PD], BF16, tag="w")
                w2b = wpool.tile([PD, NFT // 2, PD], BF16, tag="w")
                nc.sync.dma_start(
                    out=w2a, in_=r_w2[:, 0 : NFT // 2, o * PD : (o + 1) * PD]
                )
                nc.sync.dma_start(
                    out=w2b, in_=r_w2[:, NFT // 2 : NFT, o * PD : (o + 1) * PD]
                )
                for t in range(NQT):
                    tsl = slice(t * TT, (t + 1) * TT)
                    ps2 = psB.tile([PD, TT], F32, tag="b")
                    for ft in range(NFT):
                        w2_t = w2a if ft < NFT // 2 else w2b
                        nc.tensor.matmul(
                            ps2,
                            lhsT=w2_t[:, ft % (NFT // 2), :],
                            rhs=u_slice(ft, tsl),
                            start=(ft == 0),
                            stop=False,
                        )
                    nc.tensor.matmul(
                        ps2,
                        lhsT=b2_sb[:, o * PD : (o + 1) * PD],
                        rhs=onesrow[:, 0:TT],
                        start=False,
                        stop=True,
                    )
                    nc.vector.tensor_add(xq32[:, o, tsl], xq32[:, o, tsl], ps2)

            layernorm(g2_sb, be2_sb)

            # ================= output =================
            for dt in range(NDT):
                nc.sync.dma_start(out=r_yt[:, dt, :], in_=xq32[:, dt, :])

    nc.compile()
    return nc


def _get_nc():
    if "nc" not in _CACHE:
        _CACHE["nc"] = _build_nc()
    return _CACHE["nc"]


def _prep_in_maps(inputs):
    x = np.asarray(inputs["x"], np.float32)
    Wq = np.asarray(inputs["Wq"], np.float32)
    bq = np.asarray(inputs["bq"], np.float32)
    Wk = np.asarray(inputs["Wk"], np.float32)
    bk = np.asarray(inputs["bk"], np.float32)
    Wv = np.asarray(inputs["Wv"], np.float32)
    bv = np.asarray(inputs["bv"], np.float32)
    Wo = np.asarray(inputs["Wo"], np.float32)
    bo = np.asarray(inputs["bo"], np.float32)
    W1 = np.asarray(inputs["W1"], np.float32)
    b1 = np.asarray(inputs["b1"], np.float32)
    W2 = np.asarray(inputs["W2"], np.float32)
    b2 = np.asarray(inputs["b2"], np.float32)
    g1 = np.asarray(inputs["g1"], np.float32)
    be1 = np.asarray(inputs["be1"], np.float32)
    g2 = np.asarray(inputs["g2"], np.float32)
    be2 = np.asarray(inputs["be2"], np.float32)

    scale = np.float32(1.0 / np.sqrt(DH))
    bo2 = (Wo.T @ bv + bo).astype(np.float32)

    def pp(v, n):  # [n*128] -> [128, n] per-partition layout
        return np.ascontiguousarray(v.reshape(n, PD).T)

    shared = dict(
        wqbf=(Wq * scale).astype(BF),
        wkbf=Wk.astype(BF),
        wvbf=Wv.astype(BF),
        wobf=Wo.astype(BF),
        w1bf=W1.astype(BF),
        w2bf=W2.astype(BF),
        bq8=pp((bq * scale).astype(np.float32), NDT),
        bkp=pp(bk, NDT),
        b1p=pp(b1, NFT),
        bo2row=bo2.astype(BF).reshape(1, D),
        b2row=b2.astype(BF).reshape(1, D),
        g1p=pp(g1, NDT),
        be1p=pp(be1, NDT),
        g2p=pp(g2, NDT),
        be2p=pp(be2, NDT),
    )

    in_maps = []
    for c in range(8):
        b, half = c // 2, c % 2
        own = x[b, half * TQ : (half + 1) * TQ]      # [1024, 1024]
        other = x[b, (1 - half) * TQ : (2 - half) * TQ]
        ownT = np.ascontiguousarray(own.T)
        in_maps.append(
            dict(
                shared,
                xq32t=ownT,
                xqbft=ownT.astype(BF),
                xrbft=np.ascontiguousarray(other.T).astype(BF),
            )
        )
    return in_maps


def _assemble(results):
    B, S = 4, 2048
    out = np.empty((B, S, D), np.float32)
    for c in range(8):
        b, half = c // 2, c % 2
        out[b, half * TQ : (half + 1) * TQ] = results[c]["yt"].T
    return out


def _run(inputs, trace=False):
    nc = _get_nc()
    in_maps = _prep_in_maps(inputs)
    res = bass_utils.run_bass_kernel_spmd(
        nc, in_maps, core_ids=list(range(8)), trace=trace
    )
    return _assemble(res.results), res


def kernel(**inputs):
    out, _ = _run(inputs, trace=False)
    return out


def run_traced(**inputs):
    return _run(inputs, trace=True)
